# revision 23
# baseline (speedup 1.0000x reference)
"""Trainium2 Bass kernel for single-head dense attention.

Reference computation (all fp32):
    q = x @ Wq.T + bq ; k = x @ Wk.T + bk ; v = x @ Wv.T + bv      # [N, D]
    att = softmax((q @ k.T) / sqrt(128), axis=-1)                  # [N, N]
    out = (att @ v) @ Wo.T + bo + x                                # [N, D]

N = 8192, D = 1024, 8 NeuronCores.  Queries are sharded 8 ways; no
collectives needed.

Algebraic restructure (exact up to fp reassociation):
  * z = q @ k.T = (x Wq^T + bq) Wk x^T + (q . bk) 1^T.  The bk term adds a
    per-row constant, which softmax cancels exactly, so K IS NEVER
    COMPUTED.  Host folds W_qk = Wq^T Wk and b_qk = bq @ Wk; the device
    computes Q'^T = W_qk^T.T @ X_loc^T + b_qk, then S^T = X Q'^T with
    supers of X^T streamed from HBM.
  * att @ (x Wv^T + bv) Wo^T + bo = (att @ x) @ (Wo Wv)^T + (bo + Wo bv):
    the PV matmul consumes x directly (V never computed); host folds
    W_vo = Wo @ Wv and bo_eff = bo + Wo @ bv (exact: att rows sum to 1).

All four GEMM stages run in fp8e4 with perf_mode=DoubleRow (2x PE rate;
256-row contraction per matmul).  Small weights (W_qk, W_vo ~ 1e-2) are
scaled x16 on host to clear the fp8e4 subnormal floor; the x16 cancels
exactly: in the exp() scale for stage A, and via a x16 ones-vector in the
softmax denominator for the output projection.  TRN e4m3 saturates at
+-240 (not OCP 448); all staged values keep >1.7x margin.

Per-core program (Tile framework):
  warmup:  a dozen junk matmuls issued behind the initial input DMAs so
           the PE HAM clock-gate (4/8 -> 8/8 after ~3.4us busy) is warm
           when real work starts; input DMAs are chunked and dispatched
           from four engine queues in parallel to cut the startup stall.
  phase 1: Q'^T [D, 1024] fp8-DR GEMM on local tokens -> fp8 Q' planes
  phase 2: flash attention over key supers of 1024.  Stage A computes
           S^T chunks [128k, 512q] fp8-DR (keys on partitions) and exps
           them (scales folded) into fp8 P^T plane tiles.  Stage B runs
           transposed: lhsT = x-super feature chunks, rhs = P^T, giving
           O^T = (att @ x)^T [feat, q] directly -- no phase-3 transposes.
           The last super's accumulation writes O^T as fp8 DR planes.
           Denominators: a x16-ones lhsT matmul vs P^T accumulates
           16*rowsum(att) in PSUM [1, 1024q] across all supers.
  phase 3: redistribute denominators to query partitions with two tiny
           PE transposes-by-matmul, then out^T projection fp8-DR:
           out = (O^T)^T W_vo^T * (1/denom') + (x + bo_eff) fused in one
           DVE op per tile (row normalization commutes with the output
           projection; denom' = 16*denom cancels W_vo's x16).
"""

import sys

if "/opt/trn_rl_repo" not in sys.path:
    sys.path.insert(0, "/opt/trn_rl_repo")

import numpy as np

import concourse.bass as bass
import concourse.tile as tile
from concourse import bacc, mybir

N = 8192
D = 1024
NCORES = 8
TLOC = N // NCORES  # 1024 tokens per core
SCALE = float(np.sqrt(128.0))
WSCALE = 16.0       # host weight prescale (clears fp8 subnormals)
F32 = mybir.dt.float32
BF16 = mybir.dt.bfloat16
FP8 = mybir.dt.float8e4
DR = mybir.MatmulPerfMode.DoubleRow
ActF = mybir.ActivationFunctionType
AluOp = mybir.AluOpType

KSUP = 1024           # keys per attention super-block
NSUP = N // KSUP      # 8
TSUP = 512            # token block in phase 1
QBLK = 512            # query columns per S^T matmul
DC = D // 128         # 8 feature chunks
NG = DC // 2          # 4 DoubleRow 256-contraction groups
QC = TLOC // 128      # 8 query row-chunks

_PROGRAM_CACHE = {}


def build_program():
    nc = bacc.Bacc("TRN2", target_bir_lowering=False, debug=False,
                   num_devices=NCORES)

    xt_f8 = nc.dram_tensor("xt_f8", [D, N], FP8, kind="ExternalInput")
    x_f8 = nc.dram_tensor("x_f8", [N, D], FP8, kind="ExternalInput")
    xt_loc = nc.dram_tensor("xt_loc", [D, TLOC], FP8, kind="ExternalInput")
    x_loc = nc.dram_tensor("x_loc", [TLOC, D], F32, kind="ExternalInput")
    w_qk = nc.dram_tensor("w_qk", [D, D], FP8, kind="ExternalInput")
    w_vo_t = nc.dram_tensor("w_vo_t", [D, D], FP8, kind="ExternalInput")
    bqk2 = nc.dram_tensor("bqk2", [D, 1], F32, kind="ExternalInput")
    out_ext = nc.dram_tensor("out", [TLOC, D], F32, kind="ExternalOutput")
    # bounce buffer: redistributes denominators [1, q] -> [128, q/128]
    den_dram = nc.dram_tensor("den_dram", [TLOC], BF16, kind="Internal")

    # DMA dispatch is serialized per engine queue (~0.4us each); spread the
    # startup-critical input loads across the three DMA-capable queues.
    def eng(i):
        return (nc.sync, nc.gpsimd, nc.scalar)[i % 3]

    with tile.TileContext(nc) as tc:
        import contextlib

        with contextlib.ExitStack() as ctx:
            const = ctx.enter_context(tc.tile_pool(name="const", bufs=1))
            persist = ctx.enter_context(tc.tile_pool(name="persist", bufs=1))

            # [128, 2, 16] so the DR plane stride is 16B (ISA: step%16==0)
            ones_k8 = const.tile([128, 2, 16], FP8)
            nc.vector.memset(ones_k8[:], WSCALE)
            mbias = const.tile([128, 1], F32)
            nc.vector.memset(mbias[:], -3.0)
            warm_sb = const.tile([128, 2, 512], FP8)
            nc.vector.memset(warm_sb[:], 0.0)
            bqk_sb = const.tile([128, DC, 1], F32)
            nc.sync.dma_start(
                bqk_sb[:], bqk2.ap().rearrange("(c p) o -> p c o", p=128))

            # persistent SBUF tensors
            qpt_sb = persist.tile([128, DC, TLOC], FP8)      # Q'^T {ec x q}
            o_bf = persist.tile([128, DC, TLOC], BF16)       # O^T {e x q} acc
            o_f8 = persist.tile([128, DC, TLOC], FP8)        # O^T final fp8
            den16 = persist.tile([1, TLOC], BF16)            # 16*denoms
            dent_sb = persist.tile([128, QC], BF16)          # redistributed
            rden_sb = persist.tile([128, QC], F32)
            xr_sb = persist.tile([128, QC, D], F32)          # x + bo_eff
            nc.vector.memset(o_bf[:], 0.0)

            # attention pools opened before phase 1 so super-0 K/V DMAs
            # get disjoint SBUF addresses and prefetch during the Q' GEMM
            kvp = ctx.enter_context(tc.tile_pool(name="kv", bufs=2))
            ptp = ctx.enter_context(tc.tile_pool(name="pt", bufs=10))

            # ---------------- phase 1: Q'^T (local tokens) ----------------
            with nc.named_scope("p1_qproj"), \
                 tc.tile_pool(name="wqk", bufs=1) as wqkp, \
                 tc.tile_pool(name="xtl", bufs=2) as xtlp, \
                 tc.tile_pool(name="ps1", bufs=4, space="PSUM") as ps1:
                wqk_sb = wqkp.tile([128, DC, D], FP8)  # {ec x e2}
                xt0 = xtlp.tile([128, DC, TSUP], FP8, tag="xtl")
                # chunked + multi-queue so the first MMs start early
                for c in range(DC):
                    eng(c).dma_start(
                        wqk_sb[:, c, :], w_qk[c * 128:(c + 1) * 128, :])
                    eng(c + 1).dma_start(
                        xt0[:, c, :], xt_loc[c * 128:(c + 1) * 128, 0:TSUP])

                # HAM warmup: junk matmuls with no input deps fill the
                # initial DMA wait so phase 1 runs at the 8/8 clock
                for w in range(9):
                    wps = ps1.tile([128, 512], F32, tag="qp")
                    nc.tensor.matmul(
                        wps[:], lhsT=warm_sb[:, :, 0:128], rhs=warm_sb[:],
                        start=True, stop=True, perf_mode=DR)

                for ts in range(TLOC // TSUP):
                    if ts == 0:
                        xt = xt0
                    else:
                        xt = xtlp.tile([128, DC, TSUP], FP8, tag="xtl")
                        nc.sync.dma_start(
                            xt[:],
                            xt_loc[:, ts * TSUP:(ts + 1) * TSUP].rearrange(
                                "(c p) t -> p c t", p=128))
                    for dc in range(DC):
                        qp = ps1.tile([128, TSUP], F32, tag="qp")
                        for g in range(NG):
                            nc.tensor.matmul(
                                qp[:],
                                lhsT=wqk_sb[:, 2 * g:2 * g + 2,
                                            dc * 128:dc * 128 + 128],
                                rhs=xt[:, 2 * g:2 * g + 2, :],
                                start=(g == 0), stop=(g == NG - 1),
                                perf_mode=DR)
                        nc.vector.tensor_scalar_add(
                            qpt_sb[:, dc, ts * TSUP:(ts + 1) * TSUP],
                            qp[:], bqk_sb[:, dc, :])



            # ---------------- phase 2: flash attention --------------------
            with nc.named_scope("p2_attn"), \
                 tc.tile_pool(name="pso", bufs=4, space="PSUM") as pso, \
                 tc.tile_pool(name="psst", bufs=2, space="PSUM") as psst, \
                 tc.tile_pool(name="psden", bufs=2, space="PSUM") as psden:
                KC = KSUP // 128  # 8 k-chunks per super
                den_acc = persist.tile([1, TLOC], F32)  # 16*denom accumulator
                nc.vector.memset(den_acc[:], 0.0)
                for s in range(NSUP):
                    k_sb = kvp.tile([128, DC, KSUP], FP8, tag="k")
                    nc.sync.dma_start(
                        k_sb[:],
                        xt_f8[:, s * KSUP:(s + 1) * KSUP].rearrange(
                            "(c p) t -> p c t", p=128))
                    v_sb = kvp.tile([128, KSUP // 256, 2, D], FP8, tag="v")
                    nc.sync.dma_start(
                        v_sb[:],
                        x_f8[s * KSUP:(s + 1) * KSUP, :].rearrange(
                            "(g ko p) d -> p g ko d", p=128, ko=2))
                    if s == 0:
                        # residual (+bo_eff) rows for phase 3: issued after
                        # super-0's K/V loads, on the non-sync queues, so
                        # they never delay the attention-critical DMAs
                        for qc in range(QC):
                            eng(1 + qc % 2).dma_start(
                                xr_sb[:, qc, :],
                                x_loc[qc * 128:(qc + 1) * 128, :])
                    for qb in range(TLOC // QBLK):
                        # stage A: S^T chunks (fp8 DoubleRow over feature
                        # planes) -> exp(z/s - 3) -> fp8 P^T planes [Ki,Ko]
                        # (shift cancels in softmax; keeps exp under TRN
                        # e4m3 max 240)
                        pts = []
                        for kc in range(KC):
                            if kc % 2 == 0:
                                pt_t = ptp.tile([128, 2, QBLK], FP8,
                                                tag="pt")
                                pts.append(pt_t)
                            st = psst.tile([128, QBLK], F32, tag="st")
                            for g in range(NG):
                                nc.tensor.matmul(
                                    st[:],
                                    lhsT=k_sb[:, 2 * g:2 * g + 2,
                                              kc * 128:kc * 128 + 128],
                                    rhs=qpt_sb[:, 2 * g:2 * g + 2,
                                               qb * QBLK:(qb + 1) * QBLK],
                                    start=(g == 0), stop=(g == NG - 1),
                                    perf_mode=DR)
                            nc.scalar.activation(
                                pts[kc // 2][:, kc % 2, :], st[:], ActF.Exp,
                                bias=mbias[:, 0:1],
                                scale=1.0 / (WSCALE * SCALE))
                        # stage B (transposed): O^T += X_sup^T P^T, fp8 DR.
                        # lhsT = x-super feature chunks, rhs = P^T planes.
                        qsl = slice(qb * QBLK, (qb + 1) * QBLK)
                        for fc in range(DC):
                            o_ps = pso.tile([128, QBLK], F32, tag="ops")
                            for g in range(NG):
                                nc.tensor.matmul(
                                    o_ps[:],
                                    lhsT=v_sb[:, g, :,
                                              fc * 128:fc * 128 + 128],
                                    rhs=pts[g][:, :, :],
                                    start=(g == 0), stop=(g == NG - 1),
                                    perf_mode=DR)
                            if s < NSUP - 1:
                                nc.vector.tensor_add(
                                    o_bf[:, fc, qsl], o_ps[:],
                                    o_bf[:, fc, qsl])
                            else:  # final super: quantize O^T to fp8 planes
                                nc.vector.tensor_add(
                                    o_f8[:, fc, qsl], o_ps[:],
                                    o_bf[:, fc, qsl])
                        # denominators: 16*colsum(P^T) for this super,
                        # accumulated into SBUF across supers
                        d_ps = psden.tile([1, QBLK], F32, tag="dps")
                        for g in range(NG):
                            nc.tensor.matmul(
                                d_ps[0:1, :],
                                lhsT=ones_k8[:, :, 0:1],
                                rhs=pts[g][:, :, :],
                                start=(g == 0), stop=(g == NG - 1),
                                perf_mode=DR)
                        nc.vector.tensor_add(
                            den_acc[0:1, qsl], d_ps[0:1, :],
                            den_acc[0:1, qsl])
                        if s == NSUP - 1:
                            # redistribute this block's denominators
                            # [1, 512q] -> [128q, 4] via a DRAM bounce,
                            # hidden under the remaining key-loop compute
                            csl = slice(qb * 4, (qb + 1) * 4)
                            nc.vector.tensor_copy(den16[0:1, qsl],
                                                  den_acc[0:1, qsl])
                            nc.sync.dma_start(den_dram[qsl],
                                              den16[0:1, qsl])
                            nc.sync.dma_start(
                                dent_sb[:, csl],
                                den_dram[qsl].rearrange(
                                    "(c p) -> p c", p=128))
                            nc.vector.reciprocal(rden_sb[:, csl],
                                                 dent_sb[:, csl])

            # ---------------- phase 3: out-proj + normalize + residual ----
            with nc.named_scope("p3_out"), \
                 tc.tile_pool(name="wo", bufs=1) as wop, \
                 tc.tile_pool(name="fo", bufs=4) as fop, \
                 tc.tile_pool(name="psf", bufs=4, space="PSUM") as psfp:
                wo_sb = wop.tile([128, DC, D], FP8)  # {ec x d2}
                nc.sync.dma_start(
                    wo_sb[:],
                    w_vo_t.ap().rearrange("(c p) d -> p c d", p=128))

                for qc in range(QC):
                    for half in range(2):
                        fp = psfp.tile([128, 512], F32, tag="fp")
                        for g in range(NG):
                            nc.tensor.matmul(
                                fp[:],
                                lhsT=o_f8[:, 2 * g:2 * g + 2,
                                          qc * 128:(qc + 1) * 128],
                                rhs=wo_sb[:, 2 * g:2 * g + 2,
                                          half * 512:half * 512 + 512],
                                start=(g == 0), stop=(g == NG - 1),
                                perf_mode=DR)
                        fo = fop.tile([128, 512], F32, tag="fo")
                        # out = psum * (1/denom') + (x + bo_eff), fused;
                        # denom' = 16*denom cancels the x16 in W_vo
                        nc.vector.scalar_tensor_tensor(
                            fo[:], fp[:], rden_sb[:, qc:qc + 1],
                            xr_sb[:, qc, half * 512:half * 512 + 512],
                            op0=AluOp.mult, op1=AluOp.add)
                        nc.sync.dma_start(
                            out_ext[qc * 128:(qc + 1) * 128,
                                    half * 512:half * 512 + 512], fo[:])

    nc.compile()
    return nc


def _get_program():
    if "nc" not in _PROGRAM_CACHE:
        _PROGRAM_CACHE["nc"] = build_program()
    return _PROGRAM_CACHE["nc"]


def make_in_maps(x, Wq, bq, Wk, bk, Wv, bv, Wo, bo):
    """Host-side sharding/layout prep and weight folding (constant folding
    of D x D weight products -- all N-sized tensor math runs on device).
    Returns per-core input maps."""
    import ml_dtypes

    def to_f8(a):
        # TRN e4m3 saturates at +-240 (not OCP 448); clip before casting
        return np.clip(a, -240.0, 240.0).astype(ml_dtypes.float8_e4m3fn)

    x = np.ascontiguousarray(x, dtype=np.float32)
    xt = np.ascontiguousarray(x.T)
    x_f8 = to_f8(x)
    xt_f8 = to_f8(xt)
    Wq64 = np.asarray(Wq, np.float64)
    Wk64 = np.asarray(Wk, np.float64)
    Wv64 = np.asarray(Wv, np.float64)
    Wo64 = np.asarray(Wo, np.float64)
    # z = q k^T = (x Wq^T + bq) Wk x^T + (q.bk) 1^T; the bk term is a
    # per-row constant -- softmax cancels it exactly, so K is dropped.
    # x16 prescale clears the fp8 subnormal floor; cancelled in exp scale.
    w_qk = to_f8((Wq64.T @ Wk64).astype(np.float32) * WSCALE)
    bqk = (np.asarray(bq, np.float64) @ Wk64).astype(np.float32) * WSCALE
    # att(x Wv^T + bv) Wo^T + bo = (att x)(Wo Wv)^T + (bo + Wo bv),
    # exact because att rows sum to 1 in the on-device normalization.
    # x16 prescale cancelled by the x16 ones-vector in the denominator.
    w_vo_t = to_f8(np.ascontiguousarray(
        (Wo64 @ Wv64).T.astype(np.float32)) * WSCALE)
    boeff = (np.asarray(bo, np.float64)
             + Wo64 @ np.asarray(bv, np.float64)).astype(np.float32)
    in_maps = []
    for c in range(NCORES):
        sl = slice(c * TLOC, (c + 1) * TLOC)
        in_maps.append({
            "xt_f8": xt_f8,
            "x_f8": x_f8,
            "xt_loc": np.ascontiguousarray(xt_f8[:, sl]),
            "x_loc": np.ascontiguousarray(x[sl, :] + boeff[None, :]),
            "w_qk": w_qk,
            "w_vo_t": w_vo_t,
            "bqk2": bqk.reshape(D, 1),
        })
    return in_maps


def kernel(x, Wq, bq, Wk, bk, Wv, bv, Wo, bo, _trace=False):
    from concourse.bass_utils import run_bass_kernel_spmd

    nc = _get_program()
    in_maps = make_in_maps(x, Wq, bq, Wk, bk, Wv, bv, Wo, bo)
    res = run_bass_kernel_spmd(nc, in_maps, list(range(NCORES)),
                               trace=_trace)
    out = np.concatenate([res.results[c]["out"] for c in range(NCORES)],
                         axis=0)
    if _trace:
        kernel.last_results = res
    return out


# revision 24
# speedup vs baseline: 1.0368x; 1.0368x over previous
"""Trainium2 Bass kernel for single-head dense attention.

Reference computation (all fp32):
    q = x @ Wq.T + bq ; k = x @ Wk.T + bk ; v = x @ Wv.T + bv      # [N, D]
    att = softmax((q @ k.T) / sqrt(128), axis=-1)                  # [N, N]
    out = (att @ v) @ Wo.T + bo + x                                # [N, D]

N = 8192, D = 1024, 8 NeuronCores.  Queries are sharded 8 ways; no
collectives needed.

Algebraic restructure (exact up to fp reassociation):
  * z = q @ k.T = (x Wq^T + bq) Wk x^T + (q . bk) 1^T.  The bk term adds a
    per-row constant, which softmax cancels exactly, so K IS NEVER
    COMPUTED.  Host folds W_qk = Wq^T Wk and b_qk = bq @ Wk; the device
    computes Q'^T = W_qk^T.T @ X_loc^T + b_qk, then S^T = X Q'^T with
    supers of X^T streamed from HBM.
  * att @ (x Wv^T + bv) Wo^T + bo = (att @ x) @ (Wo Wv)^T + (bo + Wo bv):
    the PV matmul consumes x directly (V never computed); host folds
    W_vo = Wo @ Wv and bo_eff = bo + Wo @ bv (exact: att rows sum to 1).

All four GEMM stages run in fp8e4 with perf_mode=DoubleRow (2x PE rate;
256-row contraction per matmul).  Small weights (W_qk, W_vo ~ 1e-2) are
scaled x16 on host to clear the fp8e4 subnormal floor; the x16 cancels
exactly: in the exp() scale for stage A, and via a x16 ones-vector in the
softmax denominator for the output projection.  TRN e4m3 saturates at
+-240 (not OCP 448); all staged values keep >1.7x margin.

Per-core program (Tile framework):
  warmup:  a dozen junk matmuls issued behind the initial input DMAs so
           the PE HAM clock-gate (4/8 -> 8/8 after ~3.4us busy) is warm
           when real work starts; input DMAs are chunked and dispatched
           from four engine queues in parallel to cut the startup stall.
  phase 1: Q'^T [D, 1024] fp8-DR GEMM on local tokens -> fp8 Q' planes
  phase 2: flash attention over key supers of 1024.  Stage A computes
           S^T chunks [128k, 512q] fp8-DR (keys on partitions) and exps
           them (scales folded) into fp8 P^T plane tiles.  Stage B runs
           transposed: lhsT = x-super feature chunks, rhs = P^T, giving
           O^T = (att @ x)^T [feat, q] directly -- no phase-3 transposes.
           The last super's accumulation writes O^T as fp8 DR planes.
           Denominators: a x16-ones lhsT matmul vs P^T accumulates
           16*rowsum(att) in PSUM [1, 1024q] across all supers.
  phase 3: redistribute denominators to query partitions with two tiny
           PE transposes-by-matmul, then out^T projection fp8-DR:
           out = (O^T)^T W_vo^T * (1/denom') + (x + bo_eff) fused in one
           DVE op per tile (row normalization commutes with the output
           projection; denom' = 16*denom cancels W_vo's x16).
"""

import sys

if "/opt/trn_rl_repo" not in sys.path:
    sys.path.insert(0, "/opt/trn_rl_repo")

import numpy as np

import concourse.bass as bass
import concourse.tile as tile
from concourse import bacc, mybir

N = 8192
D = 1024
NCORES = 8
TLOC = N // NCORES  # 1024 tokens per core
SCALE = float(np.sqrt(128.0))
WSCALE = 16.0       # host weight prescale (clears fp8 subnormals)
F32 = mybir.dt.float32
BF16 = mybir.dt.bfloat16
FP8 = mybir.dt.float8e4
DR = mybir.MatmulPerfMode.DoubleRow
ActF = mybir.ActivationFunctionType
AluOp = mybir.AluOpType

KSUP = 1024           # keys per attention super-block
NSUP = N // KSUP      # 8
TSUP = 512            # token block in phase 1
QBLK = 512            # query columns per S^T matmul
DC = D // 128         # 8 feature chunks
NG = DC // 2          # 4 DoubleRow 256-contraction groups
QC = TLOC // 128      # 8 query row-chunks

_PROGRAM_CACHE = {}


def build_program():
    nc = bacc.Bacc("TRN2", target_bir_lowering=False, debug=False,
                   num_devices=NCORES)

    xt_f8 = nc.dram_tensor("xt_f8", [D, N], FP8, kind="ExternalInput")
    x_f8 = nc.dram_tensor("x_f8", [N, D], FP8, kind="ExternalInput")
    xt_loc = nc.dram_tensor("xt_loc", [D, TLOC], FP8, kind="ExternalInput")
    x_loc = nc.dram_tensor("x_loc", [TLOC, D], F32, kind="ExternalInput")
    w_qk = nc.dram_tensor("w_qk", [D, D], FP8, kind="ExternalInput")
    w_vo_t = nc.dram_tensor("w_vo_t", [D, D], FP8, kind="ExternalInput")
    bqk2 = nc.dram_tensor("bqk2", [D, 1], F32, kind="ExternalInput")
    out_ext = nc.dram_tensor("out", [TLOC, D], F32, kind="ExternalOutput")
    # bounce buffer: redistributes denominators [1, q] -> [128, q/128]
    den_dram = nc.dram_tensor("den_dram", [TLOC], BF16, kind="Internal")

    # DMA dispatch is serialized per engine queue (~0.4us each); spread the
    # startup-critical input loads across the three DMA-capable queues.
    def eng(i):
        return (nc.sync, nc.gpsimd, nc.scalar)[i % 3]

    with tile.TileContext(nc) as tc:
        import contextlib

        with contextlib.ExitStack() as ctx:
            const = ctx.enter_context(tc.tile_pool(name="const", bufs=1))
            persist = ctx.enter_context(tc.tile_pool(name="persist", bufs=1))

            # [128, 2, 16] so the DR plane stride is 16B (ISA: step%16==0)
            ones_k8 = const.tile([128, 2, 16], FP8)
            nc.vector.memset(ones_k8[:], WSCALE)
            mbias = const.tile([128, 1], F32)
            nc.vector.memset(mbias[:], -3.0)
            warm_sb = const.tile([128, 2, 512], FP8)
            nc.vector.memset(warm_sb[:], 0.0)
            bqk_sb = const.tile([128, DC, 1], F32)
            nc.sync.dma_start(
                bqk_sb[:], bqk2.ap().rearrange("(c p) o -> p c o", p=128))

            # persistent SBUF tensors
            qpt_sb = persist.tile([128, DC, TLOC], FP8)      # Q'^T {ec x q}
            o_bf = persist.tile([128, DC, TLOC], BF16)       # O^T {e x q} acc
            o_f8 = persist.tile([128, DC, TLOC], FP8)        # O^T final fp8
            den16 = persist.tile([1, TLOC], BF16)            # 16*denoms
            dent_sb = persist.tile([128, QC], BF16)          # redistributed
            rden_sb = persist.tile([128, QC], F32)
            xr_sb = persist.tile([128, QC, D], F32)          # x + bo_eff
            nc.vector.memset(o_bf[:], 0.0)

            # attention pools opened before phase 1 so super-0 K/V DMAs
            # get disjoint SBUF addresses and prefetch during the Q' GEMM
            kvp = ctx.enter_context(tc.tile_pool(name="kv", bufs=2))
            ptp = ctx.enter_context(tc.tile_pool(name="pt", bufs=10))

            # ---------------- phase 1: Q'^T (local tokens) ----------------
            with nc.named_scope("p1_qproj"), \
                 tc.tile_pool(name="wqk", bufs=1) as wqkp, \
                 tc.tile_pool(name="xtl", bufs=2) as xtlp, \
                 tc.tile_pool(name="ps1", bufs=4, space="PSUM") as ps1:
                wqk_sb = wqkp.tile([128, DC, D], FP8)  # {ec x e2}
                xt0 = xtlp.tile([128, DC, TSUP], FP8, tag="xtl")
                # chunked + multi-queue so the first MMs start early
                for c in range(DC):
                    eng(c).dma_start(
                        wqk_sb[:, c, :], w_qk[c * 128:(c + 1) * 128, :])
                    eng(c + 1).dma_start(
                        xt0[:, c, :], xt_loc[c * 128:(c + 1) * 128, 0:TSUP])

                # HAM warmup: junk matmuls with no input deps fill the
                # initial DMA wait so phase 1 runs at the 8/8 clock
                for w in range(9):
                    wps = ps1.tile([128, 512], F32, tag="qp")
                    nc.tensor.matmul(
                        wps[:], lhsT=warm_sb[:, :, 0:128], rhs=warm_sb[:],
                        start=True, stop=True, perf_mode=DR)

                for ts in range(TLOC // TSUP):
                    if ts == 0:
                        xt = xt0
                    else:
                        xt = xtlp.tile([128, DC, TSUP], FP8, tag="xtl")
                        nc.sync.dma_start(
                            xt[:],
                            xt_loc[:, ts * TSUP:(ts + 1) * TSUP].rearrange(
                                "(c p) t -> p c t", p=128))
                    for dc in range(DC):
                        qp = ps1.tile([128, TSUP], F32, tag="qp")
                        for g in range(NG):
                            nc.tensor.matmul(
                                qp[:],
                                lhsT=wqk_sb[:, 2 * g:2 * g + 2,
                                            dc * 128:dc * 128 + 128],
                                rhs=xt[:, 2 * g:2 * g + 2, :],
                                start=(g == 0), stop=(g == NG - 1),
                                perf_mode=DR)
                        nc.vector.tensor_scalar_add(
                            qpt_sb[:, dc, ts * TSUP:(ts + 1) * TSUP],
                            qp[:], bqk_sb[:, dc, :])



            # ---------------- phase 2: flash attention --------------------
            with nc.named_scope("p2_attn"), \
                 tc.tile_pool(name="pso", bufs=4, space="PSUM") as pso, \
                 tc.tile_pool(name="psst", bufs=2, space="PSUM") as psst, \
                 tc.tile_pool(name="psden", bufs=2, space="PSUM") as psden:
                KC = KSUP // 128  # 8 k-chunks per super
                den_acc = persist.tile([1, TLOC], F32)  # 16*denom accumulator
                nc.vector.memset(den_acc[:], 0.0)
                for s in range(NSUP):
                    k_sb = kvp.tile([128, DC, KSUP], FP8, tag="k")
                    nc.sync.dma_start(
                        k_sb[:],
                        xt_f8[:, s * KSUP:(s + 1) * KSUP].rearrange(
                            "(c p) t -> p c t", p=128))
                    v_sb = kvp.tile([128, KSUP // 256, 2, D], FP8, tag="v")
                    nc.sync.dma_start(
                        v_sb[:],
                        x_f8[s * KSUP:(s + 1) * KSUP, :].rearrange(
                            "(g ko p) d -> p g ko d", p=128, ko=2))
                    if s == 0:
                        # residual (+bo_eff) rows for phase 3: issued after
                        # super-0's K/V loads so they never delay the
                        # attention-critical DMAs (and off the scalar queue
                        # whose FIFO feeds the exp activations)
                        for qc in range(QC):
                            nc.sync.dma_start(
                                xr_sb[:, qc, :],
                                x_loc[qc * 128:(qc + 1) * 128, :])
                    for qb in range(TLOC // QBLK):
                        # stage A: S^T chunks (fp8 DoubleRow over feature
                        # planes) -> exp(z/s - 3) -> fp8 P^T planes [Ki,Ko]
                        # (shift cancels in softmax; keeps exp under TRN
                        # e4m3 max 240)
                        pts = []
                        for kc in range(KC):
                            if kc % 2 == 0:
                                pt_t = ptp.tile([128, 2, QBLK], FP8,
                                                tag="pt")
                                pts.append(pt_t)
                            st = psst.tile([128, QBLK], F32, tag="st")
                            for g in range(NG):
                                nc.tensor.matmul(
                                    st[:],
                                    lhsT=k_sb[:, 2 * g:2 * g + 2,
                                              kc * 128:kc * 128 + 128],
                                    rhs=qpt_sb[:, 2 * g:2 * g + 2,
                                               qb * QBLK:(qb + 1) * QBLK],
                                    start=(g == 0), stop=(g == NG - 1),
                                    perf_mode=DR)
                            nc.scalar.activation(
                                pts[kc // 2][:, kc % 2, :], st[:], ActF.Exp,
                                bias=mbias[:, 0:1],
                                scale=1.0 / (WSCALE * SCALE))
                        # stage B (transposed): O^T += X_sup^T P^T, fp8 DR.
                        # lhsT = x-super feature chunks, rhs = P^T planes.
                        qsl = slice(qb * QBLK, (qb + 1) * QBLK)
                        for fc in range(DC):
                            o_ps = pso.tile([128, QBLK], F32, tag="ops")
                            for g in range(NG):
                                nc.tensor.matmul(
                                    o_ps[:],
                                    lhsT=v_sb[:, g, :,
                                              fc * 128:fc * 128 + 128],
                                    rhs=pts[g][:, :, :],
                                    start=(g == 0), stop=(g == NG - 1),
                                    perf_mode=DR)
                            if s < NSUP - 1:
                                nc.vector.tensor_add(
                                    o_bf[:, fc, qsl], o_ps[:],
                                    o_bf[:, fc, qsl])
                            else:  # final super: quantize O^T to fp8 planes
                                nc.vector.tensor_add(
                                    o_f8[:, fc, qsl], o_ps[:],
                                    o_bf[:, fc, qsl])
                        # denominators: 16*colsum(P^T) for this super,
                        # accumulated into SBUF across supers
                        d_ps = psden.tile([1, QBLK], F32, tag="dps")
                        for g in range(NG):
                            nc.tensor.matmul(
                                d_ps[0:1, :],
                                lhsT=ones_k8[:, :, 0:1],
                                rhs=pts[g][:, :, :],
                                start=(g == 0), stop=(g == NG - 1),
                                perf_mode=DR)
                        nc.vector.tensor_add(
                            den_acc[0:1, qsl], d_ps[0:1, :],
                            den_acc[0:1, qsl])
                        if s == NSUP - 1:
                            # redistribute this block's denominators
                            # [1, 512q] -> [128q, 4] via a DRAM bounce,
                            # hidden under the remaining key-loop compute
                            csl = slice(qb * 4, (qb + 1) * 4)
                            nc.vector.tensor_copy(den16[0:1, qsl],
                                                  den_acc[0:1, qsl])
                            nc.sync.dma_start(den_dram[qsl],
                                              den16[0:1, qsl])
                            nc.sync.dma_start(
                                dent_sb[:, csl],
                                den_dram[qsl].rearrange(
                                    "(c p) -> p c", p=128))
                            nc.vector.reciprocal(rden_sb[:, csl],
                                                 dent_sb[:, csl])

            # ---------------- phase 3: out-proj + normalize + residual ----
            with nc.named_scope("p3_out"), \
                 tc.tile_pool(name="wo", bufs=1) as wop, \
                 tc.tile_pool(name="fo", bufs=4) as fop, \
                 tc.tile_pool(name="psf", bufs=4, space="PSUM") as psfp:
                wo_sb = wop.tile([128, DC, D], FP8)  # {ec x d2}
                nc.sync.dma_start(
                    wo_sb[:],
                    w_vo_t.ap().rearrange("(c p) d -> p c d", p=128))

                for qc in range(QC):
                    for half in range(2):
                        fp = psfp.tile([128, 512], F32, tag="fp")
                        for g in range(NG):
                            nc.tensor.matmul(
                                fp[:],
                                lhsT=o_f8[:, 2 * g:2 * g + 2,
                                          qc * 128:(qc + 1) * 128],
                                rhs=wo_sb[:, 2 * g:2 * g + 2,
                                          half * 512:half * 512 + 512],
                                start=(g == 0), stop=(g == NG - 1),
                                perf_mode=DR)
                        fo = fop.tile([128, 512], F32, tag="fo")
                        # out = psum * (1/denom') + (x + bo_eff), fused;
                        # denom' = 16*denom cancels the x16 in W_vo
                        nc.vector.scalar_tensor_tensor(
                            fo[:], fp[:], rden_sb[:, qc:qc + 1],
                            xr_sb[:, qc, half * 512:half * 512 + 512],
                            op0=AluOp.mult, op1=AluOp.add)
                        nc.sync.dma_start(
                            out_ext[qc * 128:(qc + 1) * 128,
                                    half * 512:half * 512 + 512], fo[:])

    nc.compile()
    return nc


def _get_program():
    if "nc" not in _PROGRAM_CACHE:
        _PROGRAM_CACHE["nc"] = build_program()
    return _PROGRAM_CACHE["nc"]


def make_in_maps(x, Wq, bq, Wk, bk, Wv, bv, Wo, bo):
    """Host-side sharding/layout prep and weight folding (constant folding
    of D x D weight products -- all N-sized tensor math runs on device).
    Returns per-core input maps."""
    import ml_dtypes

    def to_f8(a):
        # TRN e4m3 saturates at +-240 (not OCP 448); clip before casting
        return np.clip(a, -240.0, 240.0).astype(ml_dtypes.float8_e4m3fn)

    x = np.ascontiguousarray(x, dtype=np.float32)
    xt = np.ascontiguousarray(x.T)
    x_f8 = to_f8(x)
    xt_f8 = to_f8(xt)
    Wq64 = np.asarray(Wq, np.float64)
    Wk64 = np.asarray(Wk, np.float64)
    Wv64 = np.asarray(Wv, np.float64)
    Wo64 = np.asarray(Wo, np.float64)
    # z = q k^T = (x Wq^T + bq) Wk x^T + (q.bk) 1^T; the bk term is a
    # per-row constant -- softmax cancels it exactly, so K is dropped.
    # x16 prescale clears the fp8 subnormal floor; cancelled in exp scale.
    w_qk = to_f8((Wq64.T @ Wk64).astype(np.float32) * WSCALE)
    bqk = (np.asarray(bq, np.float64) @ Wk64).astype(np.float32) * WSCALE
    # att(x Wv^T + bv) Wo^T + bo = (att x)(Wo Wv)^T + (bo + Wo bv),
    # exact because att rows sum to 1 in the on-device normalization.
    # x16 prescale cancelled by the x16 ones-vector in the denominator.
    w_vo_t = to_f8(np.ascontiguousarray(
        (Wo64 @ Wv64).T.astype(np.float32)) * WSCALE)
    boeff = (np.asarray(bo, np.float64)
             + Wo64 @ np.asarray(bv, np.float64)).astype(np.float32)
    in_maps = []
    for c in range(NCORES):
        sl = slice(c * TLOC, (c + 1) * TLOC)
        in_maps.append({
            "xt_f8": xt_f8,
            "x_f8": x_f8,
            "xt_loc": np.ascontiguousarray(xt_f8[:, sl]),
            "x_loc": np.ascontiguousarray(x[sl, :] + boeff[None, :]),
            "w_qk": w_qk,
            "w_vo_t": w_vo_t,
            "bqk2": bqk.reshape(D, 1),
        })
    return in_maps


def kernel(x, Wq, bq, Wk, bk, Wv, bv, Wo, bo, _trace=False):
    from concourse.bass_utils import run_bass_kernel_spmd

    nc = _get_program()
    in_maps = make_in_maps(x, Wq, bq, Wk, bk, Wv, bv, Wo, bo)
    res = run_bass_kernel_spmd(nc, in_maps, list(range(NCORES)),
                               trace=_trace)
    out = np.concatenate([res.results[c]["out"] for c in range(NCORES)],
                         axis=0)
    if _trace:
        kernel.last_results = res
    return out
